# revision 6
# baseline (speedup 1.0000x reference)
"""Trainium2 Bass kernel for nn_AttentionBlock (4x256x64x64 self-attention).

Sharding: 8 cores = 4 batches x 2 KEY-halves. Q == K (shared projection), so
E is symmetric; each core computes, for its batch b and key half h (columns
reordered so its keys are always cols 0..2047):

  kp  = fold_bn(Wk) @ x[b] + bk'    [64, 4096]  duplicated to 128 partitions
  E   = kp[keys].T kp               [2048 j, 4096 i]   (K=64, row-packed 2x)
  T   = exp(E - n2_j)               per-partition ACT bias; n2_j = ||k_j||^2
  vt  = (x.T Wv.T) * exp(n2_j - G)  [2048 j, 256]      (G = 78, range centering)
  num = vt.T @ T                    [256, 4096] unnormalized, scaled e^{-G}
  den_j = e^{n2_j - G} * sum_i T[j, i]   (row sums == col sums by symmetry)

Host: out[:, i] = (num_h0 + num_h1)[:, i] / den_i, + bv, reshape. Per-row
scaling of softmax cancels in num/den; the diagonal (dominant) term uses the
identical stored T element in both sums, so its rounding cancels exactly.
"""

import numpy as np

import concourse.bass as bass
import concourse.bacc as bacc
import concourse.tile as tile
import concourse.mybir as mybir
from concourse.bass_utils import run_bass_kernel_spmd

B, C, HH, WW = 4, 256, 64, 64
HW = HH * WW          # 4096
CK, CV = 64, 256
P = 128
KH = HW // 2          # 2048 keys per core
NJC = KH // P         # 16 key tiles
NCORES = 8
BN_EPS = 1e-5
G = 78.0              # global exponent re-centering for num/den fp32 range

IC = 512              # i-chunk (matmul moving dim)
NS = 4                # pipeline slots (i-pairs of 1024)
KC = 512              # hw chunk for projection matmuls

F32 = mybir.dt.float32
BF16 = mybir.dt.bfloat16
EXP = mybir.ActivationFunctionType.Exp
AX = mybir.AxisListType.X
ADD = mybir.AluOpType.add
MUL = mybir.AluOpType.mult


def _emit(tc, xb, wk2, bk, wv, num_out, den_out):
    from contextlib import ExitStack

    nc = tc.nc
    with ExitStack() as ctx:
        consts = ctx.enter_context(tc.tile_pool(name="consts", bufs=1))
        big = ctx.enter_context(tc.tile_pool(name="big", bufs=1))
        work = ctx.enter_context(tc.tile_pool(name="work", bufs=4))

        # ---- constants -------------------------------------------------
        wk_sb = consts.tile([P, 2, P], BF16)
        nc.sync.dma_start(wk_sb, wk2.rearrange("(o p) c -> p o c", p=P))
        wv_sb = consts.tile([P, 2, CV], BF16)
        bk_sb = consts.tile([P, 1], F32)
        ones64 = consts.tile([CK, 1], BF16)
        nc.vector.memset(ones64, 1.0)

        # ---- big persistent SBUF tensors -------------------------------
        xb_sb = big.tile([P, 2, HW], BF16)
        kp = big.tile([P, HW], BF16)          # keys+queries, dup on part halves
        sq = big.tile([CK, KH], BF16)         # kp^2 for key columns
        n2t = big.tile([P, NJC], F32)         # ||k_j||^2, [p, jc] layout
        n2g = big.tile([P, NJC], F32)         # n2 - G
        negM = big.tile([P, NJC], F32)
        expM = big.tile([P, NJC], F32)        # exp(n2 - G)
        vt = big.tile([P, NJC, CV], BF16)     # scaled values, j on partitions
        tt = big.tile([P, NJC, HW], BF16)     # T = exp(E - n2_j)
        apart = big.tile([P, NJC, NS], F32)   # partial row sums of T
        asum = big.tile([P, NJC], F32)
        dden = big.tile([P, NJC], F32)

        xbr = xb.rearrange("(o p) f -> p o f", p=P)

        # ---- DMA: key columns first (they gate the longest chain) ------
        NXB = 8
        bs = HW // NXB
        nc.sync.dma_start(xb_sb[:, :, 0:bs], xbr[:, :, 0:bs])
        nc.sync.dma_start(bk_sb, bk)
        nc.sync.dma_start(wv_sb, wv.rearrange("(o p) c -> p o c", p=P))
        for t in range(1, NXB):
            nc.sync.dma_start(xb_sb[:, :, t * bs:(t + 1) * bs],
                              xbr[:, :, t * bs:(t + 1) * bs])

        pool_e = ctx.enter_context(
            tc.tile_pool(name="pool_e", bufs=2, space="PSUM"))

        # ---- emit helpers ----------------------------------------------
        def emit_e_group(s, g):
            """E + exp for key tiles (2g, 2g+1), i-range [s*1024, s*1024+1024).
            The two K=64 matmul streams go to different PE row groups and run
            concurrently."""
            i0 = s * 2 * IC
            jca, jcb = 2 * g, 2 * g + 1
            eta = pool_e.tile([P, 2 * IC], F32, tag="E", name=f"et_{s}_{jca}")
            etb = pool_e.tile([P, 2 * IC], F32, tag="E", name=f"et_{s}_{jcb}")
            for u in range(2):
                isl = slice(i0 + u * IC, i0 + (u + 1) * IC)
                osl = slice(u * IC, (u + 1) * IC)
                nc.tensor.matmul(eta[:, osl],
                                 lhsT=kp[0:CK, jca * P:(jca + 1) * P],
                                 rhs=kp[0:CK, isl],
                                 start=True, stop=True,
                                 tile_position=(0, 0))
                nc.tensor.matmul(etb[:, osl],
                                 lhsT=kp[CK:P, jcb * P:(jcb + 1) * P],
                                 rhs=kp[CK:P, isl],
                                 start=True, stop=True,
                                 tile_position=(CK, 0))
            for jc, et in ((jca, eta), (jcb, etb)):
                tsl = tt[:, jc, i0:i0 + 2 * IC]
                nc.scalar.activation(tsl, et, EXP, bias=negM[:, jc:jc + 1])
                nc.vector.tensor_reduce(apart[:, jc, s:s + 1], tsl,
                                        axis=AX, op=ADD)

        pv_state = {}

        def emit_pv_part(k, jcs):
            """PV accumulation for i-chunk k, key tiles jcs (both c-halves)."""
            isl = slice(k * IC, (k + 1) * IC)
            for ch in range(2):
                key = (k, ch)
                if key not in pv_state:
                    pv_state[key] = pv_pool.tile(
                        [P, IC], F32, tag=f"pv{ch}", name=f"pvps_{k}_{ch}")
                pvps = pv_state[key]
                for jc in jcs:
                    nc.tensor.matmul(pvps,
                                     lhsT=vt[:, jc, ch * P:(ch + 1) * P],
                                     rhs=tt[:, jc, isl],
                                     start=(jc == 0), stop=(jc == NJC - 1))
                if jcs[-1] == NJC - 1:
                    st = work.tile([P, IC], F32, tag=f"st{ch}",
                                   name=f"st_{k}_{ch}")
                    nc.vector.tensor_copy(st, pvps)
                    nc.sync.dma_start(num_out[ch * P:(ch + 1) * P, isl], st)
                    del pv_state[key]

        # ---- prologue: kp projection, n2, then E slot 0 and vt ---------
        with tc.tile_pool(name="pool_pre", bufs=2, space="PSUM") as pool_pre:
            for t in range(HW // KC):
                sl = slice(t * KC, (t + 1) * KC)
                ps = pool_pre.tile([P, KC], F32, tag="pre", name=f"kps_{t}")
                for o in range(2):
                    nc.tensor.matmul(ps, lhsT=wk_sb[:, o, :],
                                     rhs=xb_sb[:, o, sl],
                                     start=(o == 0), stop=(o == 1))
                nc.vector.tensor_scalar_add(kp[:, sl], ps, bk_sb)
                if t < KH // KC:
                    nc.vector.tensor_mul(sq[:, sl], kp[0:CK, sl], kp[0:CK, sl])

            # n2 per key tile: stationary = sq chunk, moving = ones -> [128,1]
            n2f = pool_pre.tile([P, KC], F32, tag="pre", name="n2ps")
            for jc in range(NJC):
                nc.tensor.matmul(n2f[:, jc:jc + 1],
                                 lhsT=sq[:, jc * P:(jc + 1) * P], rhs=ones64,
                                 start=True, stop=True)
            nc.vector.tensor_copy(n2t, n2f[:, 0:NJC])
            nc.scalar.mul(negM, n2t, -1.0)
            nc.vector.tensor_scalar_add(n2g, n2t, -G)
            nc.scalar.activation(expM, n2g, EXP)

            for g in range(NJC // 2):
                emit_e_group(0, g)

            for jc in range(NJC):
                vf = pool_pre.tile([P, KC], F32, tag="pre", name=f"vps_{jc}")
                vps = vf[:, 0:CV]
                for o in range(2):
                    nc.tensor.matmul(vps,
                                     lhsT=xb_sb[:, o, jc * P:(jc + 1) * P],
                                     rhs=wv_sb[:, o, :],
                                     start=(o == 0), stop=(o == 1))
                nc.vector.tensor_scalar_mul(vt[:, jc, :], vps,
                                            expM[:, jc:jc + 1])

        # ---- main pipeline: E slot s interleaved with PV of slot s-1 ---
        with tc.tile_pool(name="pool_pv", bufs=2, space="PSUM") as pv_pool:
            for s in range(1, NS):
                k0 = 2 * (s - 1)
                for g in range(NJC // 2):
                    emit_e_group(s, g)
                    emit_pv_part(k0, [2 * g, 2 * g + 1])
                    emit_pv_part(k0 + 1, [2 * g, 2 * g + 1])
            for g in range(NJC // 2):
                emit_pv_part(2 * (NS - 1), [2 * g, 2 * g + 1])
                emit_pv_part(2 * (NS - 1) + 1, [2 * g, 2 * g + 1])

            # ---- denominators ------------------------------------------
            nc.vector.tensor_reduce(asum, apart, axis=AX, op=ADD)
            nc.vector.tensor_mul(dden, asum, expM)
            nc.sync.dma_start(den_out, dden)


def build_nc():
    nc = bacc.Bacc(trn_type="TRN2")
    xb_d = nc.dram_tensor("xb", [C, HW], BF16, kind="ExternalInput")
    wk2_d = nc.dram_tensor("wk2", [C, P], BF16, kind="ExternalInput")
    bk_d = nc.dram_tensor("bk", [P, 1], F32, kind="ExternalInput")
    wv_d = nc.dram_tensor("wv", [C, CV], BF16, kind="ExternalInput")
    num_d = nc.dram_tensor("num", [CV, HW], F32, kind="ExternalOutput")
    den_d = nc.dram_tensor("den", [P, NJC], F32, kind="ExternalOutput")
    with tile.TileContext(nc) as tc:
        _emit(tc, xb_d[:], wk2_d[:], bk_d[:], wv_d[:], num_d[:], den_d[:])
    nc.finalize()
    return nc


_NC = None


def get_nc():
    global _NC
    if _NC is None:
        _NC = build_nc()
    return _NC


def build_in_maps(inputs):
    x = np.ascontiguousarray(np.asarray(inputs["x"], np.float32))
    Wk = np.asarray(inputs["Wk"], np.float32)
    bk = np.asarray(inputs["bk"], np.float32)
    gamma = np.asarray(inputs["bn_gamma"], np.float32)
    beta = np.asarray(inputs["bn_beta"], np.float32)
    mean = np.asarray(inputs["bn_mean"], np.float32)
    var = np.asarray(inputs["bn_var"], np.float32)
    Wv = np.asarray(inputs["Wv"], np.float32)

    inv = gamma / np.sqrt(var + BN_EPS)
    wk_eff = (inv[:, None] * Wk).astype(np.float32)
    bk_eff = (inv * bk + (beta - mean * inv)).astype(np.float32)

    import ml_dtypes
    bf = ml_dtypes.bfloat16
    wk2 = np.ascontiguousarray(
        np.concatenate([wk_eff.T, wk_eff.T], axis=1).astype(bf))   # [C, 128]
    wvT = np.ascontiguousarray(Wv.T.astype(bf))                    # [C, CV]
    bk2 = np.ascontiguousarray(
        np.concatenate([bk_eff, bk_eff]).reshape(P, 1).astype(np.float32))

    in_maps = []
    for core in range(NCORES):
        b, h = divmod(core, 2)
        xf = x[b].reshape(C, HW)
        if h == 0:
            xc = xf
        else:
            xc = np.concatenate([xf[:, KH:], xf[:, :KH]], axis=1)
        xc = np.ascontiguousarray(xc.astype(bf))
        in_maps.append({"xb": xc, "wk2": wk2, "bk": bk2, "wv": wvT})
    return in_maps


def kernel(**inputs):
    bv = np.asarray(inputs["bv"], np.float32)
    in_maps = build_in_maps(inputs)
    nc = get_nc()
    res = run_bass_kernel_spmd(nc, in_maps, core_ids=list(range(NCORES)))
    out = np.empty((B, CV, HW), np.float32)
    for b in range(B):
        n0 = np.asarray(res.results[2 * b]["num"], np.float32)
        n1 = np.asarray(res.results[2 * b + 1]["num"], np.float32)
        num = n0 + np.concatenate([n1[:, KH:], n1[:, :KH]], axis=1)
        d0 = np.asarray(res.results[2 * b]["den"], np.float32)
        d1 = np.asarray(res.results[2 * b + 1]["den"], np.float32)
        den = np.concatenate([d0.T.reshape(-1), d1.T.reshape(-1)])
        out[b] = num / den[None, :]
    out += bv[None, :, None]
    return np.ascontiguousarray(out.reshape(B, CV, HH, WW))
